# revision 8
# baseline (speedup 1.0000x reference)
"""Trainium2 Bass kernel for nn_LocalAggregator (gaussian local aggregation).

Strategy (per spec sharding hint): shard the N=15000 query points across the 8
cores (1875 each, padded to 1920); replicate the M=1200 gaussians.

Math: for each (point n, gaussian m):
  power(n,m) = -0.5 (p-mu)^T A (p-mu)   (A = inv cov) expands to a 10-dim
  dot f(n).g(m)  -> one K=10 fp32 matmul.
  The Chebyshev-box mask (integer cell logic) is computed EXACTLY via one-hot
  encodings: S(n,m) = #axes whose cell is inside the gaussian's box, obtained
  from a K=416 fp8 matmul of interval tables J against per-axis one-hots.
  Folding BIG*(S-3) into the same PSUM accumulation makes
     combined = power + BIG*(S-3)  ==  power  if in-box, else <= -BIG,
  so  alpha = exp(combined + log(opa))  needs no masking ops at all
  (exp underflows to exactly 0 outside the box; the reference's power<=0
  mask is vacuous for SPD covariances).
  Outputs per core, via PSUM-accumulated matmuls over m-tiles:
     logitsT[c,n] = sum_m sem[m,c] alpha(m,n)      (f16 matmul, lhsT=sem_aug)
     density[n]   = sum_m alpha(m,n)               (ones column of sem_aug)
     logsum[n]    = sum_m log(1-alpha(m,n))        (f16 matmul, lhsT=ones)
     bin[n]       = 1 - exp(logsum[n])
"""
import os
import sys

import numpy as np

for _p in ("/opt/trn_rl_repo", "/opt/trn_rl_repo/concourse"):
    if _p not in sys.path:
        sys.path.insert(0, _p)

import ml_dtypes
import concourse.bass as bass
import concourse.bacc as bacc
import concourse.mybir as mybir
from concourse.tile import TileContext
from concourse.bass_utils import run_bass_kernel_spmd

# problem constants (hardcoded per spec)
GRID = 0.5
PC_MIN = np.array([-50.0, -50.0, -4.0], dtype=np.float32)
SCALE_MULT = 3.0
R_MIN, R_MAX = 1, 18
GS = (200, 200, 16)          # cells per axis (H, W, D)
K_OH = sum(GS)               # 416 one-hot rows
N_TOT, M, C = 15000, 1200, 18
NCORES = 8
N_LOC = N_TOT // NCORES      # 1875
NC_W = 480                   # free-dim chunk (<=512 for one PSUM bank)
NCH = 4
N_PAD = NC_W * NCH           # 1920
M_PAD = 1280
MT = M_PAD // 128            # 10 m-tiles
BIG = 128.0                  # box penalty; exact in fp8e4, exp(-128)==0 in f32

F32 = mybir.dt.float32
F8 = mybir.dt.float8e4
F8NP = ml_dtypes.float8_e4m3
# power matmul dtype: fp32 = 4 cyc/row exact; float32r = 1 cyc/row but
# reduced multiply precision on HW (sim-identical) — gate behind env knob.
POWER_DT = (mybir.dt.float32r if os.environ.get("KERNEL_F32R")
            else mybir.dt.float32)
OUT_DT = mybir.dt.float16
OUT_NP = np.float16

KCH = [(0, 128), (128, 256), (256, 384), (384, 416)]

_prog_cache = {}


def _build_program():
    # Bacc (not raw Bass): its finalize() runs generate_event_semaphores,
    # which splits multi-wait instructions into the HW-legal <=1-wait form.
    nc = bacc.Bacc()
    fT = nc.declare_dram_parameter("fT", [10, N_PAD], POWER_DT, isOutput=False)
    ohT = nc.declare_dram_parameter("ohT", [K_OH, N_PAD], F8, isOutput=False)
    gT = nc.declare_dram_parameter("gT", [10, M_PAD], POWER_DT, isOutput=False)
    Jt = nc.declare_dram_parameter("J", [K_OH, M_PAD], F8, isOutput=False)
    semA = nc.declare_dram_parameter("semA", [128, MT * 19], OUT_DT, isOutput=False)
    logopa = nc.declare_dram_parameter("logopa", [128, MT], F32, isOutput=False)
    ones = nc.declare_dram_parameter("ones", [128, 1], OUT_DT, isOutput=False)
    outT = nc.declare_dram_parameter("outT", [20, N_PAD], F32, isOutput=True)

    AF = mybir.ActivationFunctionType
    with TileContext(nc) as tc:
        with (
            tc.tile_pool(name="const", bufs=1) as cpool,
            tc.tile_pool(name="io", bufs=2) as iopool,
            tc.tile_pool(name="work", bufs=3) as wpool,
            tc.tile_pool(name="pcomb", bufs=3, space="PSUM") as ppool,
            tc.tile_pool(name="pout", bufs=2, space="PSUM") as opool,
        ):
            # warm up the ACT table set (exp+ln) with zero-dependency dummy
            # activations so the PSEUDO_LOAD_ACT_FUNC_SET lands on an
            # instruction with no sync waits (walrus wait-slot limit).
            warm = cpool.tile([1, 8], F32, tag="warm")
            nc.vector.memset(warm[:], 0.0)
            nc.scalar.activation(warm[:], warm[:], mybir.ActivationFunctionType.Exp)
            nc.scalar.activation(warm[:], warm[:], mybir.ActivationFunctionType.Ln,
                                 bias=1.0, scale=-1.0)

            # persistent (replicated) tables
            g_s = cpool.tile([10, M_PAD], POWER_DT, tag="g")
            nc.sync.dma_start(g_s[:], gT[:])
            j_s = []
            for i, (k0, k1) in enumerate(KCH):
                t = cpool.tile([k1 - k0, M_PAD], F8, tag=f"j{i}")
                nc.sync.dma_start(t[:], Jt[k0:k1, :])
                j_s.append(t)
            sem_s = cpool.tile([128, MT * 19], OUT_DT, tag="sem")
            nc.sync.dma_start(sem_s[:], semA[:])
            lo_s = cpool.tile([128, MT], F32, tag="lopa")
            nc.sync.dma_start(lo_s[:], logopa[:])
            one_s = cpool.tile([128, 1], OUT_DT, tag="ones")
            nc.sync.dma_start(one_s[:], ones[:])

            for nch in range(NCH):
                nsl = slice(nch * NC_W, (nch + 1) * NC_W)
                f_s = iopool.tile([10, NC_W], POWER_DT, tag="f")
                nc.sync.dma_start(f_s[:], fT[:, nsl])
                oh_s = []
                for i, (k0, k1) in enumerate(KCH):
                    t = iopool.tile([k1 - k0, NC_W], F8, tag=f"oh{i}")
                    nc.sync.dma_start(t[:], ohT[k0:k1, nsl])
                    oh_s.append(t)

                ps_out = opool.tile([19, NC_W], F32, tag="out")
                ps_ls = opool.tile([1, NC_W], F32, tag="ls")

                for mt in range(MT):
                    msl = slice(mt * 128, (mt + 1) * 128)
                    ps_c = ppool.tile([128, NC_W], F32, tag="comb")
                    nc.tensor.matmul(ps_c[:], g_s[:, msl], f_s[:],
                                     start=True, stop=False)
                    for i, (k0, k1) in enumerate(KCH):
                        nc.tensor.matmul(ps_c[:], j_s[i][:, msl], oh_s[i][:],
                                         start=False, stop=(i == len(KCH) - 1))
                    alpha = wpool.tile([128, NC_W], OUT_DT, tag="alpha")
                    nc.scalar.activation(alpha[:], ps_c[:], AF.Exp,
                                         bias=lo_s[:, mt:mt + 1], scale=1.0)
                    nc.tensor.matmul(ps_out[:], sem_s[:, mt * 19:(mt + 1) * 19],
                                     alpha[:], start=(mt == 0), stop=(mt == MT - 1))
                    glog = wpool.tile([128, NC_W], OUT_DT, tag="glog")
                    # log(1 - alpha) ; alpha provably <= 0.9 here
                    nc.scalar.activation(glog[:], alpha[:], AF.Ln,
                                         bias=1.0, scale=-1.0)
                    nc.tensor.matmul(ps_ls[:], one_s[:], glog[:],
                                     start=(mt == 0), stop=(mt == MT - 1))

                # finalize chunk
                ebuf = wpool.tile([1, NC_W], F32, tag="ebuf")
                nc.scalar.activation(ebuf[:], ps_ls[:], AF.Exp)
                bin_t = wpool.tile([1, NC_W], F32, tag="bin")
                nc.vector.tensor_scalar(bin_t[:], ebuf[:], -1.0, 1.0,
                                        mybir.AluOpType.mult, mybir.AluOpType.add)
                obuf = wpool.tile([19, NC_W], F32, tag="obuf")
                nc.vector.tensor_copy(obuf[:], ps_out[:])
                nc.sync.dma_start(outT[0:19, nsl], obuf[:])
                nc.sync.dma_start(outT[19:20, nsl], bin_t[:])
    nc.finalize()
    return nc


def _host_prep(pts, means3D, opas, semantics, scales, cov3D):
    pts = np.asarray(pts, np.float32)[0]
    means = np.asarray(means3D, np.float32)[0]
    opas = np.asarray(opas, np.float32)[0]
    sem = np.asarray(semantics, np.float32)[0]
    scales = np.asarray(scales, np.float32)[0]
    cov = np.asarray(cov3D, np.float32)[0]

    pts_int = np.floor((pts - PC_MIN) / GRID).astype(np.int32)
    means_int = np.floor((means - PC_MIN) / GRID).astype(np.int32)
    radii = np.clip(np.ceil(scales.max(-1) * SCALE_MULT / GRID).astype(np.int32),
                    R_MIN, R_MAX)
    A = np.linalg.inv(cov.astype(np.float64)).astype(np.float32)
    Amu = np.einsum('mij,mj->mi', A, means).astype(np.float32)
    muAmu = np.einsum('mi,mi->m', means.astype(np.float64),
                      Amu.astype(np.float64)).astype(np.float32)

    # gaussian-side tables (replicated), padded M -> M_PAD
    g = np.zeros((10, M_PAD), np.float32)
    g[0, :M] = -0.5 * A[:, 0, 0]
    g[1, :M] = -0.5 * A[:, 1, 1]
    g[2, :M] = -0.5 * A[:, 2, 2]
    g[3, :M] = -A[:, 0, 1]
    g[4, :M] = -A[:, 0, 2]
    g[5, :M] = -A[:, 1, 2]
    g[6, :M] = Amu[:, 0]
    g[7, :M] = Amu[:, 1]
    g[8, :M] = Amu[:, 2]
    g[9, :M] = -0.5 * muAmu - 3.0 * BIG
    g[9, M:] = -3.0 * BIG  # padding gaussians: comb = -384 -> alpha = 0

    J = np.zeros((K_OH, M_PAD), F8NP)
    off = 0
    for i in range(3):
        lo = means_int[:, i] - radii
        hi = means_int[:, i] + radii
        gr = np.arange(GS[i])[:, None]
        J[off:off + GS[i], :M] = (BIG * ((gr >= lo[None, :]) &
                                         (gr <= hi[None, :]))).astype(F8NP)
        off += GS[i]

    sem_aug = np.zeros((M_PAD, 19), OUT_NP)
    sem_aug[:M, :18] = sem.astype(OUT_NP)
    sem_aug[:M, 18] = 1.0
    semA = np.zeros((128, MT * 19), OUT_NP)
    for mt in range(MT):
        semA[:, mt * 19:(mt + 1) * 19] = sem_aug[mt * 128:(mt + 1) * 128]

    logopa = np.full((M_PAD,), -1e4, np.float32)
    logopa[:M] = np.log(opas)
    lop = np.zeros((128, MT), np.float32)
    for mt in range(MT):
        lop[:, mt] = logopa[mt * 128:(mt + 1) * 128]

    ones = np.ones((128, 1), OUT_NP)

    # per-point (sharded) tables
    in_maps = []
    for c in range(NCORES):
        sl = slice(c * N_LOC, (c + 1) * N_LOC)
        p = pts[sl]
        pi = pts_int[sl]
        # pad with copies of the last point (outputs discarded)
        p = np.concatenate([p, np.repeat(p[-1:], N_PAD - N_LOC, 0)], 0)
        pi = np.concatenate([pi, np.repeat(pi[-1:], N_PAD - N_LOC, 0)], 0)
        f = np.stack([p[:, 0] ** 2, p[:, 1] ** 2, p[:, 2] ** 2,
                      p[:, 0] * p[:, 1], p[:, 0] * p[:, 2], p[:, 1] * p[:, 2],
                      p[:, 0], p[:, 1], p[:, 2],
                      np.ones(N_PAD, np.float32)], 0).astype(np.float32)
        oh = np.zeros((K_OH, N_PAD), F8NP)
        off = 0
        for i in range(3):
            gr = np.arange(GS[i])[:, None]
            oh[off:off + GS[i], :] = (gr == pi[:, i][None, :]).astype(F8NP)
            off += GS[i]
        in_maps.append({
            "fT": f, "ohT": np.ascontiguousarray(oh),
            "gT": g, "J": np.ascontiguousarray(J), "semA": semA,
            "logopa": lop, "ones": ones,
        })
    return in_maps


def kernel(pts, means3D, opas, semantics, scales, cov3D):
    if "nc" not in _prog_cache:
        _prog_cache["nc"] = _build_program()
    nc = _prog_cache["nc"]
    in_maps = _host_prep(pts, means3D, opas, semantics, scales, cov3D)
    trace = bool(os.environ.get("KERNEL_TRACE"))
    if trace:
        try:
            br = run_bass_kernel_spmd(nc, in_maps, list(range(NCORES)),
                                      trace=True)
            _prog_cache["last_exec_time_ns"] = br.exec_time_ns
        except Exception:
            # NTFF profiling hook unavailable in this container — fall back
            # to an untraced run.
            _prog_cache["last_exec_time_ns"] = None
            br = run_bass_kernel_spmd(nc, in_maps, list(range(NCORES)))
    else:
        br = run_bass_kernel_spmd(nc, in_maps, list(range(NCORES)))
    res = br.results

    logits = np.empty((N_TOT, C), np.float32)
    bin_l = np.empty((N_TOT,), np.float32)
    dens = np.empty((N_TOT,), np.float32)
    for c in range(NCORES):
        o = np.asarray(res[c]["outT"], np.float32)[:, :N_LOC]
        sl = slice(c * N_LOC, (c + 1) * N_LOC)
        logits[sl] = o[0:18].T
        dens[sl] = o[18]
        bin_l[sl] = o[19]
    return logits, bin_l, dens
